# revision 1
# baseline (speedup 1.0000x reference)
"""Trainium2 Bass kernel for EquidistantDiscreteContinuousConv3d.

Math: out = conv3d(x, einsum('ogk,kzyx->ogzyx', weight, psi_local), stride 2,
pad 2) + bias, with x [2,8,128,128,128] -> out [2,16,64,64,64].

The dense 5^3 kernel only has taps within Euclidean radius 2 (33 of 125
offsets are nonzero). Sharding: 8 cores = batch(2) x z-groups(4); each core
computes out[b, :, 16g:16g+16] from an overlapping, zero-padded input slab.
No collectives — halos materialize as overlapping host-side slices.

Device mapping: the tensor engine contracts K = (z_local(16) x ic(8)) = 128
partitions, with M = (oz_sub x oc(16)) packed into a block-banded weight
matrix (band encodes the 5 dz taps, zeros elsewhere), looped over the 13
(dy, dx) stencil taps that accumulate in PSUM. rhs slices come from a
phase-decomposed (even/odd y and x, de-interleaved so the innermost 64
x-positions are contiguous) view of the input tile. Inputs arrive as 12
y-halved sub-units, each as two overlapping half-DMAs, so the first matmul
starts after ~1 MB of DMA and transfers stay at full bandwidth (at most two
in flight, same-kind halves never concurrent so per-kind semaphore counts
are completion-exact).

Raw Bacc pipeline per core (static, fully unrolled; no TileContext):
  SP  : 24 paced input half-DMAs (xt slot = unit%5), then end-of-run sem clear
  ACT : wtile DMA, then 12 half-stage output DMAs
  PE  : 24 groups x 13 banded matmuls accumulating in psum bank g%8
  DVE : 24 psum->stage copies (stage slot = s%2)
"""

import os

import ml_dtypes
import numpy as np

BF16 = ml_dtypes.bfloat16

IC, OC = 8, 16
TAPS_XY = [
    (dy, dx) for dy in range(-2, 3) for dx in range(-2, 3) if dy * dy + dx * dx <= 4
]  # 13 taps
OZ_PER = (6, 6, 4)
SUB_FREE = 36 * 132  # y-half sub-unit free size: (yo 18, yp 2, px 2, xe 66)
N_CORES = 8

_MODULE = None
LAST_RESULT = None  # BassKernelResults of the most recent run (for test harness)


def _build_module():
    from contextlib import ExitStack

    import concourse.bacc as bacc
    import concourse.mybir as mybir

    f32 = mybir.dt.float32
    bf16 = mybir.dt.bfloat16

    nc = bacc.Bacc()
    x_in = nc.dram_tensor("xc", [12, 128, SUB_FREE], bf16, kind="ExternalInput")
    w_in = nc.dram_tensor("wc", [128, 13 * 128], bf16, kind="ExternalInput")
    out = nc.dram_tensor("out", [16, 16, 64, 64], f32, kind="ExternalOutput")

    NG = 24  # groups: g = (((c*2)+h)*2+q)*2+tt
    NSLOT = 5
    ROW = 2 * 2 * 66  # one yo row = (yp, px, xe) block of 264 elements

    def gdec(g):
        c, r = divmod(g, 8)
        h, r = divmod(r, 4)
        q, tt = divmod(r, 2)
        return c, h, q, tt

    with ExitStack() as ctx:
        wsem = ctx.enter_context(nc.semaphore("wsem"))
        xsA = [ctx.enter_context(nc.semaphore(f"xsemA{i}")) for i in range(2)]
        xsB = [ctx.enter_context(nc.semaphore(f"xsemB{i}")) for i in range(2)]
        pesem = ctx.enter_context(nc.semaphore("pesem"))
        dvsem = ctx.enter_context(nc.semaphore("dvsem"))
        osem = ctx.enter_context(nc.semaphore("osem"))
        wtile = ctx.enter_context(nc.sbuf_tensor("wtile", [128, 13 * 128], bf16))
        xts = [
            ctx.enter_context(nc.sbuf_tensor(f"xt{i}", [128, SUB_FREE], bf16))
            for i in range(NSLOT)
        ]
        stgs = [
            ctx.enter_context(nc.sbuf_tensor(f"stg{i}", [128, 4 * 512], f32))
            for i in range(2)
        ]
        pss = [
            ctx.enter_context(nc.psum_tensor(f"ps{i}", [128, 512], f32))
            for i in range(8)
        ]
        x5s = [
            t[:].rearrange("p (a b d c) -> p a b d c", a=18, b=2, d=2, c=66)
            for t in xts
        ]

        with nc.Block() as block:

            @block.sync
            def _(sp):
                # half A = yo [0,10) (enough for the tt=0 group); A halves all
                # ride the SP queue, B halves the ACT queue. Each queue
                # alternates between two sems so receipts overlap transfers
                # while same-sem counts stay completion-exact.
                for i in range(12):
                    if i == 1:
                        sp.wait_ge(xsA[0], 16)  # let A0 land at full BW
                    elif i >= 2:
                        sp.wait_ge(xsA[i % 2], 16 * (i // 2))
                    if i >= NSLOT:
                        sp.wait_ge(pesem, 2 * (i - NSLOT) + 2)  # slot free
                    sp.dma_start(
                        out=xts[i % NSLOT][:, 0 : 10 * ROW],
                        in_=x_in[i, :, 0 : 10 * ROW],
                    ).then_inc(xsA[i % 2], 16)
                # re-execution safety: clear sems once everything is done
                sp.wait_ge(osem, 16 * 24)  # all 24 out DMAs done
                for sem in (wsem, xsA[0], xsA[1], xsB[0], xsB[1], pesem, dvsem, osem):
                    sp.sem_clear(sem)

            @block.scalar
            def _(act):
                act.dma_start(out=wtile[:], in_=w_in[:]).then_inc(wsem, 16)

                def bdma(i):
                    if i == 0:
                        pass  # B0 shares the ramp with A0/wtile
                    elif i == 1:
                        act.wait_ge(xsB[0], 16)
                    else:
                        act.wait_ge(xsB[i % 2], 16 * (i // 2))
                    if i >= NSLOT:
                        act.wait_ge(pesem, 2 * (i - NSLOT) + 2)
                    act.dma_start(
                        out=xts[i % NSLOT][:, 8 * ROW : 18 * ROW],
                        in_=x_in[i, :, 8 * ROW : 18 * ROW],
                    ).then_inc(xsB[i % 2], 16)

                def odma(s, t):
                    c, h = divmod(s, 2)
                    M = OZ_PER[c] * 16
                    act.wait_ge(dvsem, 4 * s + t + 1)  # copy (s,t) done
                    dst = out[
                        6 * c : 6 * c + OZ_PER[c],
                        :,
                        32 * h + 8 * t : 32 * h + 8 * t + 8,
                        :,
                    ].rearrange("a b c d -> (a b) (c d)")
                    act.dma_start(
                        out=dst, in_=stgs[s % 2][:M, 512 * t : 512 * t + 512]
                    ).then_inc(osem, 16)

                # interleave B-half inputs with output quarters so neither
                # starves: B(ui) is needed well after out(s) waits clear
                for i in range(5):
                    bdma(i)
                k = 5
                for s in range(6):
                    for t in range(4):
                        odma(s, t)
                        if k < 12 and t % 2 == 1:
                            bdma(k)
                            k += 1

            @block.tensor
            def _(pe):
                # HAM warm-up: cheap N=64 throwaway matmuls keep PE busy from
                # the preamble until the first input lands, so the clock gate
                # is at 2.4 GHz for every real matmul. Inputs may be mid-DMA
                # garbage; psum bank 7 is discarded by its first start=True.
                for _ in range(96):
                    pe.matmul(
                        pss[7][:, 0:64], wtile[:, 0:128], wtile[:, 0:64],
                        start=True, stop=True,
                    )
                pe.wait_ge(wsem, 16)
                for g in range(NG):
                    c, h, q, tt = gdec(g)
                    i = g // 2
                    pe.wait_ge(xsA[i % 2], 16 * (i // 2 + 1))
                    if tt == 1:
                        pe.wait_ge(xsB[i % 2], 16 * (i // 2 + 1))
                    if g >= 8:
                        pe.wait_ge(dvsem, g - 7)  # psum bank g%8 evacuated
                    x5 = x5s[i % NSLOT]
                    ps = pss[g % 8]
                    for j, (dy, dx) in enumerate(TAPS_XY):
                        jy, py = divmod(dy + 2, 2)
                        jx, px = divmod(dx + 2, 2)
                        a0 = 8 * tt + jy
                        rhs = x5[
                            :, a0 : a0 + 8, py : py + 1, px : px + 1, jx : jx + 64
                        ]
                        mm = pe.matmul(
                            ps[:],
                            wtile[:, j * 128 : (j + 1) * 128],
                            rhs,
                            start=(j == 0),
                            stop=(j == len(TAPS_XY) - 1),
                        )
                        if j == len(TAPS_XY) - 1:
                            mm.then_inc(pesem, 1)

            @block.vector
            def _(dve):
                for g in range(NG):
                    s = g // 4
                    t = g % 4
                    M = OZ_PER[g // 8] * 16
                    if t == 0 and s >= 2:
                        dve.wait_ge(osem, 64 * (s - 1))  # stage slot s%2 free
                    dve.wait_ge(pesem, g + 1)
                    dve.tensor_copy(
                        out=stgs[s % 2][:M, t * 512 : (t + 1) * 512],
                        in_=pss[g % 8][:M],
                    ).then_inc(dvsem, 1)

    nc.compile()
    return nc


def _get_module():
    global _MODULE
    if _MODULE is None:
        _MODULE = _build_module()
    return _MODULE


def _band_weights(w5):
    """wc[k=(z*8+ic), j*128 + ozs*16 + oc] block-banded weights (ozs 6,7 pad)."""
    wc = np.zeros((128, 13, 8, 16), np.float32)
    for j, (dy, dx) in enumerate(TAPS_XY):
        for dzi in range(5):
            dz = dzi - 2
            if dz * dz + dy * dy + dx * dx > 4:
                continue
            blk = w5[:, :, dzi, dy + 2, dx + 2].T  # [ic, oc]
            for ozs in range(6):
                z = 2 * ozs + dzi
                wc[z * 8 : (z + 1) * 8, j, ozs, :] = blk
    return np.ascontiguousarray(wc.reshape(128, 13 * 128))


def _shard_core_input(x, b, gz):
    """Per-core padded input as 12 y-halved sub-units [128, 36*132]."""
    xp = np.zeros((IC, 40, 132, 132), BF16)
    z_lo = 32 * gz - 2
    src_lo, src_hi = max(0, z_lo), min(128, z_lo + 40)
    xp[:, src_lo - z_lo : src_hi - z_lo, 2:130, 2:130] = x[b, :, src_lo:src_hi, :, :]
    units = np.empty((12, 128, SUB_FREE), BF16)
    for c in range(3):
        for h in range(2):
            for q in range(2):
                u = xp[:, 12 * c : 12 * c + 16, 64 * h + 32 * q : 64 * h + 32 * q + 36, :]
                # de-interleave phases: free = (yo 18, yp 2, px 2, xe 66)
                u = u.reshape(IC, 16, 36, 66, 2).transpose(0, 1, 2, 4, 3)
                u = u.reshape(IC, 16, 18, 2, 2, 66)
                units[c * 4 + h * 2 + q] = u.transpose(1, 0, 2, 3, 4, 5).reshape(
                    128, SUB_FREE
                )
    return units


def kernel(x, weight, bias, psi_local):
    global LAST_RESULT
    from concourse.bass_utils import run_bass_kernel_spmd

    x = np.asarray(x, np.float32)
    weight = np.asarray(weight, np.float32)
    bias = np.asarray(bias, np.float32)
    psi_local = np.asarray(psi_local, np.float32)

    w5 = np.einsum("ogk,kzyx->ogzyx", weight, psi_local).astype(np.float32)
    wc = _band_weights(w5).astype(BF16)

    in_maps = []
    for core in range(N_CORES):
        b, gz = divmod(core, 4)
        in_maps.append({"xc": _shard_core_input(x, b, gz), "wc": wc})

    nc = _get_module()
    trace = bool(int(os.environ.get("KERNEL_TRACE", "0")))
    res = run_bass_kernel_spmd(
        nc, in_maps, core_ids=list(range(N_CORES)), trace=trace
    )
    LAST_RESULT = res

    out = np.empty((2, OC, 64, 64, 64), np.float32)
    for core in range(N_CORES):
        b, gz = divmod(core, 4)
        out[b, :, 16 * gz : 16 * gz + 16] = res.results[core]["out"].transpose(
            1, 0, 2, 3
        )
    out += bias[None, :, None, None, None]
    return out



# revision 7
# speedup vs baseline: 1.0912x; 1.0912x over previous
"""Trainium2 Bass kernel for EquidistantDiscreteContinuousConv3d.

Math: out = conv3d(x, einsum('ogk,kzyx->ogzyx', weight, psi_local), stride 2,
pad 2) + bias, with x [2,8,128,128,128] -> out [2,16,64,64,64].

The dense 5^3 kernel only has taps within Euclidean radius 2 (33 of 125
offsets are nonzero). Sharding: 8 cores = batch(2) x y-quarters(4); each core
computes out[b, :, :, 16q:16q+16] from an overlapping, zero-padded input
slab. No collectives -- halos materialize as overlapping host-side slices.

Device mapping: the tensor engine contracts K = (z_window(16) x ic(8)) = 128
partitions, with M = (oz_sub x oc(16)) packed into a block-banded weight
matrix (band encodes the dz taps), looped over the 13 (dy, dx) stencil taps
that accumulate in PSUM. Giving each core the FULL z extent (64 oz) makes
the 6-oz-per-16-plane-window blocking waste only boundary slots. Four lead
units cover 7 oz each via a shifted window [2*oz0-1, +16) plus a one-matmul
"fixup" (the single missing (oz0, dz=-2, dy=0, dx=0) combo) that contracts
8 K-rows of the PREVIOUS unit's tile; the remaining six units are plain
6-oz windows. 10 units x 2 oy-halves = 20 PSUM groups, 266 matmuls of
N=512 vs 312 for a uniform-6 z-sharded layout.

rhs slices come from a phase-decomposed (even/odd y and x, de-interleaved so
the innermost 64 x-positions are contiguous) view of the input tile. Each
unit arrives as two parallel half-DMAs (yo rows 0..9 on the SP queue, rows
10..17 on the ACT queue) so tile 0 lands fast; wtile and the 10 output DMAs
ride the otherwise-idle GPSIMD queue. Output is written bf16 (cast on the
psum->stage copy) and widened to f32 on the host.

Raw Bacc pipeline per core (static, fully unrolled; no TileContext):
  SP    : 10 paced input A-half DMAs + even-unit output DMAs, end sem clear
  ACT   : wtile DMA, 10 paced input B-half DMAs + odd-unit output DMAs
  PE    : short HAM warm-up, then 20 groups x (13|14) banded matmuls
  DVE   : 20 psum->stage bf16 copies (stage slot = unit%2)
"""

import os

import ml_dtypes
import numpy as np

BF16 = ml_dtypes.bfloat16

IC, OC = 8, 16
TAPS_XY = [
    (dy, dx) for dy in range(-2, 3) for dx in range(-2, 3) if dy * dy + dx * dx <= 4
]  # 13 taps
SUB_FREE = 36 * 132  # per-unit free size: (yo 18, yp 2, px 2, xe 66)
ROW = 2 * 2 * 66  # one yo row = (yp, px, xe) block of 264 elements
N_CORES = 8
NSLOT = 6
WARMUP = 32

# units: (oz0, noz, zp0, wkind, fixup) -- zp = z + 2 (host pad), window is
# zp [zp0, zp0+16). wkind 0: band rel plane = 2*ozs+dzi (6-oz); wkind 1:
# rel = 2*ozs+dzi-1 (7-oz, shifted window).
UNITS = [(7 * u, 7, 14 * u + 1, 1, u > 0) for u in range(4)] + [
    (28 + 6 * k, 6, 56 + 12 * k, 0, False) for k in range(6)
]
NU = len(UNITS)
NW = 27  # weight tile: 13 (6-oz band) + 13 (7-oz band) + 1 (fixup)
ZP = 132  # padded z planes on host (z = zp - 2)

_MODULE = None
LAST_RESULT = None  # BassKernelResults of the most recent run (for test harness)


def _build_module():
    from contextlib import ExitStack

    import concourse.bacc as bacc
    import concourse.mybir as mybir

    f32 = mybir.dt.float32
    bf16 = mybir.dt.bfloat16

    nc = bacc.Bacc()
    x_in = nc.dram_tensor("xc", [NU, 128, SUB_FREE], bf16, kind="ExternalInput")
    w_in = nc.dram_tensor("wc", [128, NW * 128], bf16, kind="ExternalInput")
    out = nc.dram_tensor("out", [64, 16, 16, 64], bf16, kind="ExternalOutput")

    with ExitStack() as ctx:
        wsem = ctx.enter_context(nc.semaphore("wsem"))
        xsA = [ctx.enter_context(nc.semaphore(f"xsemA{i}")) for i in range(2)]
        xsB = [ctx.enter_context(nc.semaphore(f"xsemB{i}")) for i in range(2)]
        pesem = ctx.enter_context(nc.semaphore("pesem"))
        dvsem = ctx.enter_context(nc.semaphore("dvsem"))
        oss = [ctx.enter_context(nc.semaphore(f"osem{i}")) for i in range(2)]
        wtile = ctx.enter_context(nc.sbuf_tensor("wtile", [128, NW * 128], bf16))
        xts = [
            ctx.enter_context(nc.sbuf_tensor(f"xt{i}", [128, SUB_FREE], bf16))
            for i in range(NSLOT)
        ]
        stgs = [
            ctx.enter_context(nc.sbuf_tensor(f"stg{i}", [128, 2 * 512], bf16))
            for i in range(2)
        ]
        pss = [
            ctx.enter_context(nc.psum_tensor(f"ps{i}", [128, 512], f32))
            for i in range(8)
        ]
        x5s = [
            t[:].rearrange("p (a b d c) -> p a b d c", a=18, b=2, d=2, c=66)
            for t in xts
        ]

        def odma(eng, u):
            oz0, noz = UNITS[u][0], UNITS[u][1]
            eng.wait_ge(dvsem, 2 * u + 2)  # both copies staged
            dst = out[oz0 : oz0 + noz, :, :, :].rearrange(
                "a b c d -> (a b) (c d)"
            )
            eng.dma_start(
                out=dst, in_=stgs[u % 2][: 16 * noz, :]
            ).then_inc(oss[u % 2], 16)

        with nc.Block() as block:

            @block.sync
            def _(sp):
                # A half = yo rows [0,10) (all that oy-half tt=0 needs). A
                # halves ride the SP queue, B halves the ACT queue; each
                # queue alternates two sems so receipts overlap transfers
                # while same-sem counts stay completion-exact. Even-unit
                # output DMAs interleave on SP behind the input stream.
                def adma(u):
                    if u == 1:
                        sp.wait_ge(xsA[0], 16)  # let A0 land at full BW
                    elif u >= 2:
                        sp.wait_ge(xsA[u % 2], 16 * (u // 2))
                    if u >= NSLOT:
                        # tile slot free once unit (u-NSLOT+1) -- the fixup
                        # reader of tile u-NSLOT -- has retired both groups
                        sp.wait_ge(pesem, 2 * (u - NSLOT) + 4)
                    sp.dma_start(
                        out=xts[u % NSLOT][:, 0 : 10 * ROW],
                        in_=x_in[u, :, 0 : 10 * ROW],
                    ).then_inc(xsA[u % 2], 16)

                for u in range(NSLOT + 1):
                    adma(u)
                k = 0
                for u in range(NSLOT + 1, NU):
                    odma(sp, k)
                    k += 2
                    adma(u)
                while k < NU:
                    odma(sp, k)
                    k += 2
                # re-execution safety: wait+clear each sem at its final
                # count (waits fuse FIFO into the next instruction, so each
                # clear directly carries its own sem's final-value wait; the
                # oss pairs double as the end-of-run barrier)
                nA, nB = (NU + 1) // 2, NU // 2
                for sem, v in (
                    (oss[0], 16 * nA), (oss[1], 16 * nB), (wsem, 16),
                    (xsA[0], 16 * nA), (xsA[1], 16 * nB),
                    (xsB[0], 16 * nA), (xsB[1], 16 * nB),
                    (pesem, 2 * NU), (dvsem, 2 * NU),
                ):
                    sp.wait_ge(sem, v)
                    sp.sem_clear(sem)

            @block.scalar
            def _(act):
                act.dma_start(out=wtile[:], in_=w_in[:]).then_inc(wsem, 16)

                def bdma(u):
                    if u == 1:
                        act.wait_ge(xsB[0], 16)
                    elif u >= 2:
                        act.wait_ge(xsB[u % 2], 16 * (u // 2))
                    if u >= NSLOT:
                        act.wait_ge(pesem, 2 * (u - NSLOT) + 4)
                    act.dma_start(
                        out=xts[u % NSLOT][:, 10 * ROW : 18 * ROW],
                        in_=x_in[u, :, 10 * ROW : 18 * ROW],
                    ).then_inc(xsB[u % 2], 16)

                for u in range(NSLOT + 1):
                    bdma(u)
                k = 1
                for u in range(NSLOT + 1, NU):
                    odma(act, k)
                    k += 2
                    bdma(u)
                while k < NU:
                    odma(act, k)
                    k += 2

            @block.tensor
            def _(pe):
                # Short HAM warm-up on garbage: keeps PE busy while tile 0
                # and wtile stream in, so most real matmuls run at 2.4 GHz.
                # psum bank 7 is discarded by its first start=True.
                for _ in range(WARMUP):
                    pe.matmul(
                        pss[7][:, 0:64], wtile[:, 0:128], wtile[:, 0:64],
                        start=True, stop=True,
                    )
                pe.wait_ge(wsem, 16)
                g = 0
                for u, (oz0, noz, zp0, wkind, fixup) in enumerate(UNITS):
                    for tt in range(2):
                        pe.wait_ge(xsA[u % 2], 16 * (u // 2 + 1))
                        if tt == 1:
                            pe.wait_ge(xsB[u % 2], 16 * (u // 2 + 1))
                        if g >= 8:
                            pe.wait_ge(dvsem, g - 7)  # psum bank g%8 evacuated
                        ps = pss[g % 8]
                        if fixup:
                            # (oz0, dz=-2, dy=0, dx=0) from the previous tile
                            x5p = x5s[(u - 1) % NSLOT]
                            pe.matmul(
                                ps[:],
                                wtile[:, 26 * 128 : 27 * 128],
                                x5p[:, 8 * tt + 1 : 8 * tt + 9, 0:1, 0:1, 1:65],
                                start=True, stop=False,
                            )
                        x5 = x5s[u % NSLOT]
                        for j, (dy, dx) in enumerate(TAPS_XY):
                            jy, py = divmod(dy + 2, 2)
                            jx, px = divmod(dx + 2, 2)
                            a0 = 8 * tt + jy
                            rhs = x5[
                                :, a0 : a0 + 8, py : py + 1, px : px + 1, jx : jx + 64
                            ]
                            c0 = (13 * wkind + j) * 128
                            mm = pe.matmul(
                                ps[:],
                                wtile[:, c0 : c0 + 128],
                                rhs,
                                start=(j == 0 and not fixup),
                                stop=(j == len(TAPS_XY) - 1),
                            )
                            if j == len(TAPS_XY) - 1:
                                mm.then_inc(pesem, 1)
                        g += 1

            @block.vector
            def _(dve):
                for g in range(2 * NU):
                    u, tt = divmod(g, 2)
                    M = 16 * UNITS[u][1]
                    if tt == 0 and u >= 2:
                        # stage slot u%2 free: same-parity odmas are serialized,
                        # so the per-parity count is completion-exact
                        dve.wait_ge(oss[u % 2], 16 * (u // 2))
                    dve.wait_ge(pesem, g + 1)
                    dve.tensor_copy(
                        out=stgs[u % 2][:M, 512 * tt : 512 * tt + 512],
                        in_=pss[g % 8][:M],
                    ).then_inc(dvsem, 1)

    nc.compile()
    return nc


def _get_module():
    global _MODULE
    if _MODULE is None:
        _MODULE = _build_module()
    return _MODULE


def _band_weights(w5):
    """wc[k=(z*8+ic), (13*wkind+j)*128 + ozs*16 + oc] block-banded weights.

    wkind 0: 6-oz window, rel plane = 2*ozs+dzi. wkind 1: 7-oz shifted
    window, rel = 2*ozs+dzi-1 (the z=-1 miss is the fixup's job). Column
    block 26 is the fixup matrix: tap (dz=-2, dy=0, dx=0) for ozs 0 read
    from the previous tile's rel plane 13.
    """
    wc = np.zeros((128, NW, 128), np.float32)
    for j, (dy, dx) in enumerate(TAPS_XY):
        for dzi in range(5):
            dz = dzi - 2
            if dz * dz + dy * dy + dx * dx > 4:
                continue
            blk = w5[:, :, dzi, dy + 2, dx + 2].T  # [ic, oc]
            for ozs in range(6):
                z = 2 * ozs + dzi
                wc[z * 8 : (z + 1) * 8, j, ozs * 16 : ozs * 16 + 16] = blk
            for ozs in range(7):
                z = 2 * ozs + dzi - 1
                if 0 <= z < 16:
                    wc[z * 8 : (z + 1) * 8, 13 + j, ozs * 16 : ozs * 16 + 16] = blk
    wc[13 * 8 : 14 * 8, 26, 0:16] = w5[:, :, 0, 2, 2].T
    return np.ascontiguousarray(wc.reshape(128, NW * 128))


def _shard_core_input(x, b, q):
    """Per-core padded input as NU z-window units [128, 36*132]."""
    xp = np.zeros((IC, ZP, 36, 132), BF16)
    y_lo = 32 * q - 2
    ys_lo, ys_hi = max(0, y_lo), min(128, y_lo + 36)
    xp[:, 2:130, ys_lo - y_lo : ys_hi - y_lo, 2:130] = x[
        b, :, :, ys_lo:ys_hi, :
    ]
    units = np.empty((NU, 128, SUB_FREE), BF16)
    for i, (_, _, zp0, _, _) in enumerate(UNITS):
        u = xp[:, zp0 : zp0 + 16]
        # de-interleave phases: free = (yo 18, yp 2, px 2, xe 66)
        u = u.reshape(IC, 16, 36, 66, 2).transpose(0, 1, 2, 4, 3)
        u = u.reshape(IC, 16, 18, 2, 2, 66)
        units[i] = u.transpose(1, 0, 2, 3, 4, 5).reshape(128, SUB_FREE)
    return units


def kernel(x, weight, bias, psi_local):
    global LAST_RESULT
    from concourse.bass_utils import run_bass_kernel_spmd

    x = np.asarray(x, np.float32)
    weight = np.asarray(weight, np.float32)
    bias = np.asarray(bias, np.float32)
    psi_local = np.asarray(psi_local, np.float32)

    w5 = np.einsum("ogk,kzyx->ogzyx", weight, psi_local).astype(np.float32)
    wc = _band_weights(w5).astype(BF16)

    in_maps = []
    for core in range(N_CORES):
        b, q = divmod(core, 4)
        in_maps.append({"xc": _shard_core_input(x, b, q), "wc": wc})

    nc = _get_module()
    trace = bool(int(os.environ.get("KERNEL_TRACE", "0")))
    res = run_bass_kernel_spmd(
        nc, in_maps, core_ids=list(range(N_CORES)), trace=trace
    )
    LAST_RESULT = res

    out = np.empty((2, OC, 64, 64, 64), np.float32)
    for core in range(N_CORES):
        b, q = divmod(core, 4)
        out[b, :, :, 16 * q : 16 * q + 16, :] = (
            res.results[core]["out"].astype(np.float32).transpose(1, 0, 2, 3)
        )
    out += bias[None, :, None, None, None]
    return out


# revision 8
# speedup vs baseline: 1.0949x; 1.0034x over previous
"""Trainium2 Bass kernel for EquidistantDiscreteContinuousConv3d.

Math: out = conv3d(x, einsum('ogk,kzyx->ogzyx', weight, psi_local), stride 2,
pad 2) + bias, with x [2,8,128,128,128] -> out [2,16,64,64,64].

The dense 5^3 kernel only has taps within Euclidean radius 2 (33 of 125
offsets are nonzero). Sharding: 8 cores = batch(2) x y-quarters(4); each core
computes out[b, :, :, 16q:16q+16] from an overlapping, zero-padded input
slab. No collectives -- halos materialize as overlapping host-side slices.

Device mapping: the tensor engine contracts K = (z_window(16) x ic(8)) = 128
partitions, with M = (oz_sub x oc(16)) packed into a block-banded weight
matrix (band encodes the dz taps), looped over the 13 (dy, dx) stencil taps
that accumulate in PSUM. Giving each core the FULL z extent (64 oz) makes
the 6-oz-per-16-plane-window blocking waste only boundary slots. Four lead
units cover 7 oz each via a shifted window [2*oz0-1, +16) plus a one-matmul
"fixup" (the single missing (oz0, dz=-2, dy=0, dx=0) combo) that contracts
8 K-rows of the PREVIOUS unit's tile; the remaining six units are plain
6-oz windows. 10 units x 2 oy-halves = 20 PSUM groups, 266 matmuls of
N=512 vs 312 for a uniform-6 z-sharded layout.

rhs slices come from a phase-decomposed (even/odd y and x, de-interleaved so
the innermost 64 x-positions are contiguous) view of the input tile. Each
unit arrives as two parallel half-DMAs (yo rows 0..9 on the SP queue, rows
10..17 on the ACT queue) so tile 0 lands fast; wtile and the 10 output DMAs
ride the otherwise-idle GPSIMD queue. Output is written bf16 (cast on the
psum->stage copy) and widened to f32 on the host.

Raw Bacc pipeline per core (static, fully unrolled; no TileContext):
  SP    : 10 paced input A-half DMAs + even-unit output DMAs, end sem clear
  ACT   : wtile DMA, 10 paced input B-half DMAs + odd-unit output DMAs
  PE    : short HAM warm-up, then 20 groups x (13|14) banded matmuls
  DVE   : 20 psum->stage bf16 copies (stage slot = unit%2)
"""

import os

import ml_dtypes
import numpy as np

BF16 = ml_dtypes.bfloat16

IC, OC = 8, 16
TAPS_XY = [
    (dy, dx) for dy in range(-2, 3) for dx in range(-2, 3) if dy * dy + dx * dx <= 4
]  # 13 taps
SUB_FREE = 36 * 132  # per-unit free size: (yo 18, yp 2, px 2, xe 66)
ROW = 2 * 2 * 66  # one yo row = (yp, px, xe) block of 264 elements
N_CORES = 8
NSLOT = 6
WARMUP = 80

# units: (oz0, noz, zp0, wkind, fixup) -- zp = z + 2 (host pad), window is
# zp [zp0, zp0+16). wkind 0: band rel plane = 2*ozs+dzi (6-oz); wkind 1:
# rel = 2*ozs+dzi-1 (7-oz, shifted window).
UNITS = [(7 * u, 7, 14 * u + 1, 1, u > 0) for u in range(4)] + [
    (28 + 6 * k, 6, 56 + 12 * k, 0, False) for k in range(6)
]
NU = len(UNITS)
NW = 27  # weight tile: 13 (6-oz band) + 13 (7-oz band) + 1 (fixup)
ZP = 132  # padded z planes on host (z = zp - 2)

_MODULE = None
LAST_RESULT = None  # BassKernelResults of the most recent run (for test harness)


def _build_module():
    from contextlib import ExitStack

    import concourse.bacc as bacc
    import concourse.mybir as mybir

    f32 = mybir.dt.float32
    bf16 = mybir.dt.bfloat16

    nc = bacc.Bacc()
    x_in = nc.dram_tensor("xc", [NU, 128, SUB_FREE], bf16, kind="ExternalInput")
    w_in = nc.dram_tensor("wc", [128, NW * 128], bf16, kind="ExternalInput")
    out = nc.dram_tensor("out", [64, 16, 2, 8, 64], bf16, kind="ExternalOutput")

    with ExitStack() as ctx:
        wsem = ctx.enter_context(nc.semaphore("wsem"))
        xsA = [ctx.enter_context(nc.semaphore(f"xsemA{i}")) for i in range(2)]
        xsB = [ctx.enter_context(nc.semaphore(f"xsemB{i}")) for i in range(2)]
        pesem = ctx.enter_context(nc.semaphore("pesem"))
        dvsem = ctx.enter_context(nc.semaphore("dvsem"))
        oss = [ctx.enter_context(nc.semaphore(f"osem{i}")) for i in range(2)]
        wtile = ctx.enter_context(nc.sbuf_tensor("wtile", [128, NW * 128], bf16))
        xts = [
            ctx.enter_context(nc.sbuf_tensor(f"xt{i}", [128, SUB_FREE], bf16))
            for i in range(NSLOT)
        ]
        stgs = [
            ctx.enter_context(nc.sbuf_tensor(f"stg{i}", [128, 2 * 512], bf16))
            for i in range(2)
        ]
        pss = [
            ctx.enter_context(nc.psum_tensor(f"ps{i}", [128, 512], f32))
            for i in range(8)
        ]
        x5s = [
            t[:].rearrange("p (a b d c) -> p a b d c", a=18, b=2, d=2, c=66)
            for t in xts
        ]

        def odma(eng, u):
            oz0, noz = UNITS[u][0], UNITS[u][1]
            if u < NU - 1:
                eng.wait_ge(dvsem, 2 * u + 2)  # both copies staged
                dst = out[oz0 : oz0 + noz, :, :, :, :].rearrange(
                    "a b c d e -> (a b) (c d e)"
                )
                eng.dma_start(
                    out=dst, in_=stgs[u % 2][: 16 * noz, :]
                ).then_inc(oss[u % 2], 16)
            else:
                # last unit ships per-half so the final transfer after the
                # last psum evacuation is small (short kernel tail)
                for tt in range(2):
                    eng.wait_ge(dvsem, 2 * u + tt + 1)
                    dst = out[oz0 : oz0 + noz, :, tt : tt + 1, :, :].rearrange(
                        "a b c d e -> (a b) (c d e)"
                    )
                    eng.dma_start(
                        out=dst,
                        in_=stgs[u % 2][: 16 * noz, 512 * tt : 512 * tt + 512],
                    ).then_inc(oss[u % 2], 16)

        with nc.Block() as block:

            @block.sync
            def _(sp):
                # A half = yo rows [0,10) (all that oy-half tt=0 needs). A
                # halves ride the SP queue, B halves the ACT queue; each
                # queue alternates two sems so receipts overlap transfers
                # while same-sem counts stay completion-exact. Even-unit
                # output DMAs interleave on SP behind the input stream.
                def adma(u):
                    if u == 1:
                        sp.wait_ge(xsA[0], 16)  # let A0 land at full BW
                    elif u >= 2:
                        sp.wait_ge(xsA[u % 2], 16 * (u // 2))
                    if u >= NSLOT:
                        # tile slot free once unit (u-NSLOT+1) -- the fixup
                        # reader of tile u-NSLOT -- has retired both groups
                        sp.wait_ge(pesem, 2 * (u - NSLOT) + 4)
                    sp.dma_start(
                        out=xts[u % NSLOT][:, 0 : 10 * ROW],
                        in_=x_in[u, :, 0 : 10 * ROW],
                    ).then_inc(xsA[u % 2], 16)

                adma(0)
                # first-needed weight half rides SP right behind A0 (HWDGE
                # rings are FIFO per issuing engine), so the stream can
                # start without waiting for the 6-oz bands
                sp.dma_start(
                    out=wtile[:, 13 * 128 :], in_=w_in[:, 13 * 128 :]
                ).then_inc(wsem, 16)
                for u in range(1, NSLOT + 1):
                    adma(u)
                k = 0
                for u in range(NSLOT + 1, NU):
                    odma(sp, k)
                    k += 2
                    adma(u)
                while k < NU:
                    odma(sp, k)
                    k += 2
                # re-execution safety: the oss totals are the end-of-run
                # barrier (all other sems are transitively final by then),
                # then clear everything
                sp.wait_ge(oss[0], 16 * ((NU + 1) // 2))
                sp.wait_ge(oss[1], 16 * (NU // 2 + 1))
                for sem in (
                    wsem, xsA[0], xsA[1], xsB[0], xsB[1], pesem, dvsem,
                    oss[0], oss[1],
                ):
                    sp.sem_clear(sem)

            @block.scalar
            def _(act):
                def bdma(u):
                    if u == 1:
                        act.wait_ge(xsB[0], 16)
                    elif u >= 2:
                        act.wait_ge(xsB[u % 2], 16 * (u // 2))
                    if u >= NSLOT:
                        act.wait_ge(pesem, 2 * (u - NSLOT) + 4)
                    act.dma_start(
                        out=xts[u % NSLOT][:, 10 * ROW : 18 * ROW],
                        in_=x_in[u, :, 10 * ROW : 18 * ROW],
                    ).then_inc(xsB[u % 2], 16)

                bdma(0)
                bdma(1)
                act.dma_start(
                    out=wtile[:, : 13 * 128], in_=w_in[:, : 13 * 128]
                ).then_inc(wsem, 16)  # 6-oz bands, first needed at unit 4
                for u in range(2, NSLOT + 1):
                    bdma(u)
                k = 1
                for u in range(NSLOT + 1, NU):
                    odma(act, k)
                    k += 2
                    bdma(u)
                while k < NU:
                    odma(act, k)
                    k += 2

            @block.tensor
            def _(pe):
                # Short HAM warm-up on garbage: keeps PE busy while tile 0
                # and wtile stream in, so most real matmuls run at 2.4 GHz.
                # psum bank 7 is discarded by its first start=True.
                for _ in range(WARMUP):
                    pe.matmul(
                        pss[7][:, 0:64], wtile[:, 0:128], wtile[:, 0:64],
                        start=True, stop=True,
                    )
                pe.wait_ge(wsem, 16)
                g = 0
                for u, (oz0, noz, zp0, wkind, fixup) in enumerate(UNITS):
                    if u == 4:
                        pe.wait_ge(wsem, 32)  # 6-oz bands landed
                    for tt in range(2):
                        pe.wait_ge(xsA[u % 2], 16 * (u // 2 + 1))
                        if tt == 1:
                            pe.wait_ge(xsB[u % 2], 16 * (u // 2 + 1))
                        if g >= 8:
                            pe.wait_ge(dvsem, g - 7)  # psum bank g%8 evacuated
                        ps = pss[g % 8]
                        if fixup:
                            # (oz0, dz=-2, dy=0, dx=0) from the previous tile
                            x5p = x5s[(u - 1) % NSLOT]
                            pe.matmul(
                                ps[:],
                                wtile[:, 26 * 128 : 27 * 128],
                                x5p[:, 8 * tt + 1 : 8 * tt + 9, 0:1, 0:1, 1:65],
                                start=True, stop=False,
                            )
                        x5 = x5s[u % NSLOT]
                        for j, (dy, dx) in enumerate(TAPS_XY):
                            jy, py = divmod(dy + 2, 2)
                            jx, px = divmod(dx + 2, 2)
                            a0 = 8 * tt + jy
                            rhs = x5[
                                :, a0 : a0 + 8, py : py + 1, px : px + 1, jx : jx + 64
                            ]
                            c0 = (13 * wkind + j) * 128
                            mm = pe.matmul(
                                ps[:],
                                wtile[:, c0 : c0 + 128],
                                rhs,
                                start=(j == 0 and not fixup),
                                stop=(j == len(TAPS_XY) - 1),
                            )
                            if j == len(TAPS_XY) - 1:
                                mm.then_inc(pesem, 1)
                        g += 1

            @block.vector
            def _(dve):
                for g in range(2 * NU):
                    u, tt = divmod(g, 2)
                    M = 16 * UNITS[u][1]
                    if tt == 0 and u >= 2:
                        # stage slot u%2 free: same-parity odmas are serialized,
                        # so the per-parity count is completion-exact
                        dve.wait_ge(oss[u % 2], 16 * (u // 2))
                    dve.wait_ge(pesem, g + 1)
                    dve.tensor_copy(
                        out=stgs[u % 2][:M, 512 * tt : 512 * tt + 512],
                        in_=pss[g % 8][:M],
                    ).then_inc(dvsem, 1)

    nc.compile()
    return nc


def _get_module():
    global _MODULE
    if _MODULE is None:
        _MODULE = _build_module()
    return _MODULE


def _band_weights(w5):
    """wc[k=(z*8+ic), (13*wkind+j)*128 + ozs*16 + oc] block-banded weights.

    wkind 0: 6-oz window, rel plane = 2*ozs+dzi. wkind 1: 7-oz shifted
    window, rel = 2*ozs+dzi-1 (the z=-1 miss is the fixup's job). Column
    block 26 is the fixup matrix: tap (dz=-2, dy=0, dx=0) for ozs 0 read
    from the previous tile's rel plane 13.
    """
    wc = np.zeros((128, NW, 128), np.float32)
    for j, (dy, dx) in enumerate(TAPS_XY):
        for dzi in range(5):
            dz = dzi - 2
            if dz * dz + dy * dy + dx * dx > 4:
                continue
            blk = w5[:, :, dzi, dy + 2, dx + 2].T  # [ic, oc]
            for ozs in range(6):
                z = 2 * ozs + dzi
                wc[z * 8 : (z + 1) * 8, j, ozs * 16 : ozs * 16 + 16] = blk
            for ozs in range(7):
                z = 2 * ozs + dzi - 1
                if 0 <= z < 16:
                    wc[z * 8 : (z + 1) * 8, 13 + j, ozs * 16 : ozs * 16 + 16] = blk
    wc[13 * 8 : 14 * 8, 26, 0:16] = w5[:, :, 0, 2, 2].T
    return np.ascontiguousarray(wc.reshape(128, NW * 128))


def _shard_core_input(x, b, q):
    """Per-core padded input as NU z-window units [128, 36*132]."""
    xp = np.zeros((IC, ZP, 36, 132), BF16)
    y_lo = 32 * q - 2
    ys_lo, ys_hi = max(0, y_lo), min(128, y_lo + 36)
    xp[:, 2:130, ys_lo - y_lo : ys_hi - y_lo, 2:130] = x[
        b, :, :, ys_lo:ys_hi, :
    ]
    units = np.empty((NU, 128, SUB_FREE), BF16)
    for i, (_, _, zp0, _, _) in enumerate(UNITS):
        u = xp[:, zp0 : zp0 + 16]
        # de-interleave phases: free = (yo 18, yp 2, px 2, xe 66)
        u = u.reshape(IC, 16, 36, 66, 2).transpose(0, 1, 2, 4, 3)
        u = u.reshape(IC, 16, 18, 2, 2, 66)
        units[i] = u.transpose(1, 0, 2, 3, 4, 5).reshape(128, SUB_FREE)
    return units


def kernel(x, weight, bias, psi_local):
    global LAST_RESULT
    from concourse.bass_utils import run_bass_kernel_spmd

    x = np.asarray(x, np.float32)
    weight = np.asarray(weight, np.float32)
    bias = np.asarray(bias, np.float32)
    psi_local = np.asarray(psi_local, np.float32)

    w5 = np.einsum("ogk,kzyx->ogzyx", weight, psi_local).astype(np.float32)
    wc = _band_weights(w5).astype(BF16)

    in_maps = []
    for core in range(N_CORES):
        b, q = divmod(core, 4)
        in_maps.append({"xc": _shard_core_input(x, b, q), "wc": wc})

    nc = _get_module()
    trace = bool(int(os.environ.get("KERNEL_TRACE", "0")))
    res = run_bass_kernel_spmd(
        nc, in_maps, core_ids=list(range(N_CORES)), trace=trace
    )
    LAST_RESULT = res

    out = np.empty((2, OC, 64, 64, 64), np.float32)
    for core in range(N_CORES):
        b, q = divmod(core, 4)
        co = res.results[core]["out"].astype(np.float32).reshape(64, 16, 16, 64)
        out[b, :, :, 16 * q : 16 * q + 16, :] = co.transpose(1, 0, 2, 3)
    out += bias[None, :, None, None, None]
    return out
